# revision 1
# baseline (speedup 1.0000x reference)
"""Stein solver  Lambda - A @ Lambda @ W = C @ Y  on 8 trn2 NeuronCores.

Math: Lambda = sum_k A^k R W^k with R = C@Y; contraction ||A||2*||W||2 ~ 0.32.
Smith doubling truncated at 4 terms, then 2 exact fixed-point iterations:
    S1 = R + (A C) Y W           (2 terms; U0 = (A C) Y avoids gathering R)
    S2 = S1 + A^2 S1 W^2         (4 terms; ~1%-magnitude term, bf16 GEMMs)
    Sp = R + A S W   (x2, fp32)  (polish: each contracts error ~12x -> ~5e-7)

Distribution: row-sharded over 8 cores, core c owns rows [128c, 128c+128).
Stationary operand = transposed own-shard (8 k-tiles of [128,128]); moving
operand = the full matrix streamed from DRAM (W's split form stays SBUF-
resident for the polish).  A^2/W^2 run FIRST (input-only deps) so their
AllGather hides behind all of phase 1; S1/S2/Sp AllGathers are split into
column halves so the consumer GEMM starts on half 0 while half 1 flies.

Precision tiers, chosen by measured per-instruction cost (each matmul pays
a ~300ns weight-load gap, so fewer/wider passes win):
  f32r  (~13 mantissa bits, 1 HW inst/GEMM): V, U0, S1 pre-polish chain.
  bf16  (1 inst, half DMA): A^2, W^2, U1, S2 -- 1%-scale terms.
  split-bf16 (hi+lo pairs, 3 insts, ~17-18 bits): both polish iterations.
  fp32  (2 slow insts, exact): only R = C@Y, which enters the answer 1:1.
AllGather payloads: S1 bf16 (consumer is a 1%-scale term), S2/Sp split-bf16
pairs.  Complex GEMM = 4 real GEMMs with the real-part subtraction folded
into PSUM accumulation via pre-negated imag weights; each polish iteration
contracts the remaining error by the spectral factor ~0.08 -> ~6e-7 final.
"""

import numpy as np

P = 128
N = 1024
KT = N // P          # 8 k-tiles
NC = 8               # cores
NCH = 2              # 512-wide n-chunks per 1024-col output row block

_compiled = {}


def _build():
    import concourse.mybir as mybir
    import concourse.tile as tile
    from concourse import bacc
    from concourse.masks import make_identity

    f32 = mybir.dt.float32
    bf16 = mybir.dt.bfloat16

    nc = bacc.Bacc("TRN2", target_bir_lowering=False, debug=False, num_devices=NC)

    # ---- I/O ----  full matrices laid out [partition, plane, ktile, col]:
    # X[kt*128+p, c] at [p, j, kt, c]; shards [partition, (re,im,-im), ktile, m]
    f32r = mybir.dt.float32r
    Cfull = nc.dram_tensor("Cfull", [P, 2, KT, N], f32r, kind="ExternalInput")
    Wfull = nc.dram_tensor("Wfull", [P, 2, KT, N], f32r, kind="ExternalInput")
    ATshr = nc.dram_tensor("ATshr", [P, 3, KT, P], f32r, kind="ExternalInput")
    Wsp_d = nc.dram_tensor("Wsp_d", [P, 4, KT, N], bf16, kind="ExternalInput")
    Yfull32 = nc.dram_tensor("Yfull32", [P, 2, KT, N], f32, kind="ExternalInput")
    Afull_bf = nc.dram_tensor("Afull_bf", [P, 2, KT, N], bf16, kind="ExternalInput")
    Wfull_bf = nc.dram_tensor("Wfull_bf", [P, 2, KT, N], bf16, kind="ExternalInput")
    ATsp = nc.dram_tensor("ATsp", [P, 6, KT, P], bf16, kind="ExternalInput")
    ATsh_bf = nc.dram_tensor("ATsh_bf", [P, 3, KT, P], bf16, kind="ExternalInput")
    WTsh_bf = nc.dram_tensor("WTsh_bf", [P, 3, KT, P], bf16, kind="ExternalInput")
    CTsh32 = nc.dram_tensor("CTsh32", [P, 3, KT, P], f32, kind="ExternalInput")
    out = nc.dram_tensor("out", [2, P, N], f32, kind="ExternalOutput")

    RG = [list(range(NC))]

    with tile.TileContext(nc) as tc:
        with (
            tc.tile_pool(name="wpin", bufs=1) as wpin,        # pinned
            tc.tile_pool(name="wrot", bufs=2) as wrot,        # rotating weights
            tc.tile_pool(name="rhs", bufs=3) as rpool,        # rhs stream tiles
            tc.tile_pool(name="acc", bufs=2) as apool,        # shard accumulators
            tc.tile_pool(name="psum", bufs=6, space="PSUM") as ppool,
            tc.tile_pool(name="tpsum", bufs=2, space="PSUM") as tppool,
            tc.tile_pool(name="dram", bufs=1, space="DRAM") as dram,
        ):
            ident = wpin.tile([P, P], f32, tag="ident")
            make_identity(nc, ident)
            ident_bf = wpin.tile([P, P], bf16, tag="identbf")
            nc.vector.tensor_copy(ident_bf[:], ident[:])

            ATspw = wpin.tile([P, 6, KT, P], bf16, tag="ATspw")
            nc.sync.dma_start(ATspw[:], ATsp.ap())

            def load_weights(dram_t, tag, dtype, pool=wrot, bufs=None):
                wt = pool.tile([P, 3, KT, P], dtype, tag=tag, name="wt_" + tag,
                               bufs=bufs)
                nc.sync.dma_start(wt[:], dram_t.ap())
                return wt

            def cgemm(XT, rhs_slice, out_cb, dtype, resident=None, halves=None):
                """out(own 128 rows x 1024, complex) = own_rows(X) @ M.

                XT: [P,3,KT,P] weights (re, im, -im).  Moving operand: either
                rhs_slice(j,t) -> DRAM [P,N] AP (streamed via SBUF tiles) or
                resident(j,t) -> SBUF [P,N] AP.  out_cb(j, ci, psum).
                """
                ps = [[ppool.tile([P, 512], f32, tag="ps", name="ps")
                       for _ in range(NCH)] for _ in range(2)]

                def do4(ci, rsl, cs, st, sp):
                    nc.tensor.matmul(ps[0][ci][:], XT[:, 0, t], rsl(0, cs), start=st, stop=False)
                    nc.tensor.matmul(ps[0][ci][:], XT[:, 2, t], rsl(1, cs), start=False, stop=sp)
                    nc.tensor.matmul(ps[1][ci][:], XT[:, 0, t], rsl(1, cs), start=st, stop=False)
                    nc.tensor.matmul(ps[1][ci][:], XT[:, 1, t], rsl(0, cs), start=False, stop=sp)

                if halves is not None:
                    for ci in range(NCH):
                        for t in range(KT):
                            rt = rpool.tile([P, 2, 512], dtype, tag="rhsh", name="rth", bufs=3)
                            nc.sync.dma_start(rt[:], halves[ci](t))
                            do4(ci, (lambda j, cs, rt=rt: rt[:, j, :]), None, t == 0, t == KT - 1)
                        for j in range(2):
                            out_cb(j, ci, ps[j][ci])
                    return
                else:
                    for t in range(KT):
                        if resident is None:
                            rt = rpool.tile([P, 2, N], dtype, tag="rhs", name="rt")
                            nc.sync.dma_start(rt[:, 0], rhs_slice(0, t))
                            nc.sync.dma_start(rt[:, 1], rhs_slice(1, t))
                            rsl = lambda j, cs: rt[:, j, cs]
                        else:
                            rsl = lambda j, cs, t=t: resident(j, t)[:, cs]
                        st = t == 0
                        sp = t == KT - 1
                        for ci in range(NCH):
                            cs = slice(512 * ci, 512 * ci + 512)
                            do4(ci, rsl, cs, st, sp)
                for j in range(2):
                    for ci in range(NCH):
                        out_cb(j, ci, ps[j][ci])

            def cgemm_sp(XT6, rhs_slice, out_cb, resident=None, halves=None):
                """Split-bf16 complex GEMM: X, M given as hi/lo bf16 pairs.

                XT6: [P,6,KT,P] weights (rh, rl, ih, il, -ih, -il).
                rhs_slice(t) -> DRAM [P,4,N] AP (planes rh, rl, ih, il), or
                resident(y, t) -> SBUF [P,N] AP, or halves[ci](t) -> [P,4,512]
                column-half APs (chunk-outer order, for pipelined AllGathers).
                """
                MMS_R = ((0, 0), (0, 1), (1, 0), (4, 2), (4, 3), (5, 2))
                MMS_I = ((0, 2), (0, 3), (1, 2), (2, 0), (2, 1), (3, 0))
                ps = [[ppool.tile([P, 512], f32, tag="ps", name="ps")
                       for _ in range(NCH)] for _ in range(2)]

                def do_mms(ci, rsl, cs, st, sp):
                    for k, (w, y) in enumerate(MMS_R):
                        nc.tensor.matmul(ps[0][ci][:], XT6[:, w, t], rsl(y, cs),
                                         start=st and k == 0, stop=sp and k == 5)
                    for k, (w, y) in enumerate(MMS_I):
                        nc.tensor.matmul(ps[1][ci][:], XT6[:, w, t], rsl(y, cs),
                                         start=st and k == 0, stop=sp and k == 5)

                if halves is not None:
                    for ci in range(NCH):
                        for t in range(KT):
                            rt = rpool.tile([P, 4, 512], bf16, tag="rhsh", name="rth", bufs=3)
                            nc.sync.dma_start(rt[:], halves[ci](t))
                            rsl = lambda y, cs, rt=rt: rt[:, y, :]
                            do_mms(ci, rsl, None, t == 0, t == KT - 1)
                        for j in range(2):
                            out_cb(j, ci, ps[j][ci])
                    return
                else:
                    for t in range(KT):
                        if resident is None:
                            rt = rpool.tile([P, 4, N], bf16, tag="rhs", name="rt")
                            nc.sync.dma_start(rt[:], rhs_slice(t))
                            rsl = lambda y, cs: rt[:, y, cs]
                        else:
                            rsl = lambda y, cs, t=t: resident(y, t)[:, cs]
                        st = t == 0
                        sp = t == KT - 1
                        for ci in range(NCH):
                            cs = slice(512 * ci, 512 * ci + 512)
                            do_mms(ci, rsl, cs, st, sp)
                for j in range(2):
                    for ci in range(NCH):
                        out_cb(j, ci, ps[j][ci])

            def transpose_to_weights_sp(src, tag, pool=wrot, bufs=None):
                """fp32 [P, 2, N] shard tile -> [P,6,KT,P] split-bf16 weights."""
                wt = pool.tile([P, 6, KT, P], bf16, tag=tag, name="tsp_" + tag,
                               bufs=bufs)
                for j in range(2):
                    for t in range(KT):
                        tp = tppool.tile([P, P], f32, tag="tp", name="tp")
                        nc.tensor.transpose(tp[:], src[:, j, 128 * t:128 * t + 128], ident[:])
                        h = 2 * j
                        nc.vector.tensor_copy(wt[:, h, t], tp[:])
                        nc.vector.tensor_sub(wt[:, h + 1, t], tp[:], wt[:, h, t])
                        if j == 1:
                            nc.vector.tensor_scalar_mul(wt[:, 4, t], tp[:], -1.0)
                            nc.vector.tensor_sub(wt[:, 5, t], wt[:, 2, t], tp[:])
                return wt

            def transpose_to_weights(src, tag, dtype, pool=wrot, bufs=None):
                """[P, 2, N] shard tile -> [P,3,KT,P] transposed weights."""
                wt = pool.tile([P, 3, KT, P], dtype, tag=tag, name="tw_" + tag,
                               bufs=bufs)
                bf = src.dtype == bf16
                for j in range(2):
                    for t in range(KT):
                        tp = tppool.tile([P, P], bf16 if bf else f32, tag="tp", name="tp")
                        blk = src[:, j, 128 * t:128 * t + 128]
                        if bf:
                            nc.tensor.transpose(tp[:], blk, ident_bf[:])
                        else:
                            nc.tensor.transpose(tp[:], blk, ident[:])
                        nc.vector.tensor_copy(wt[:, j, t], tp[:])
                        if j == 1:
                            nc.vector.tensor_scalar_mul(wt[:, 2, t], tp[:], -1.0)
                return wt

            def cb_store(dst):
                def cb(j, ci, psum):
                    nc.vector.tensor_copy(dst[:, j, 512 * ci:512 * ci + 512], psum[:])
                return cb

            def allgather(ag_in, ag_out):
                nc.gpsimd.collective_compute(
                    "AllGather", mybir.AluOpType.bypass, replica_groups=RG,
                    ins=[ag_in.opt()], outs=[ag_out.opt()],
                )

            def src_of(dram_t):
                ap = dram_t.ap()
                return lambda j, t: ap[:, j, t]

            wspa = Wsp_d.ap()
            wsp_halves = [lambda t, ci=ci: wspa[:, :, t, 512 * ci:512 * ci + 512]
                          for ci in range(NCH)]

            # ---------------- squares first: A1 = A^2, W1 = W^2 (bf16) ----
            # input-only deps, so their AllGather flies behind all of phase 1
            ATbf = load_weights(ATsh_bf, tag="T3", dtype=bf16, bufs=1)
            WTbf = load_weights(WTsh_bf, tag="T2", dtype=bf16, bufs=1)
            A1 = apool.tile([P, 2, N], bf16, tag="work", bufs=3, name="A1")
            cgemm(ATbf, src_of(Afull_bf), cb_store(A1), bf16)

            aga_in = dram.tile([P, 2, N], bf16, name="aga_in")
            aga_out = dram.tile([NC, P, 2, N], bf16, addr_space="Shared", name="aga_out")
            W1 = apool.tile([P, 2, N], bf16, tag="work", bufs=3, name="W1")

            def cb_w1(j, ci, psum):
                cs = slice(512 * ci, 512 * ci + 512)
                nc.vector.tensor_copy(W1[:, j, cs], psum[:])
                nc.sync.dma_start(aga_in[:, j, cs], W1[:, j, cs])

            cgemm(WTbf, src_of(Wfull_bf), cb_w1, bf16)
            allgather(aga_in, aga_out)
            AT1 = transpose_to_weights(A1, tag="T3", dtype=bf16, bufs=1)

            # ---------------- phase 1 (no collective deps) ----------------
            CT32 = load_weights(CTsh32, tag="T2", dtype=f32, bufs=1)

            # V = A @ C  (f32r)
            ATw = load_weights(ATshr, tag="T1", dtype=f32r)
            V = apool.tile([P, 2, N], f32, tag="work", bufs=3, name="V")
            cgemm(ATw, src_of(Cfull), cb_store(V), f32r)

            VT = transpose_to_weights(V, tag="T1", dtype=f32)

            # Fused R = C@Y (fp32) and U0 = V@Y (f32r) over ONE Y stream.
            # The f32r-typed tile serves both: RHS reads a bitcast-fp32 view
            # (legal; the reverse direction is not), U0 reads it natively.
            R32 = apool.tile([P, 2, N], f32, tag="R32", bufs=1)
            U0 = apool.tile([P, 2, N], f32, tag="work", bufs=3, name="U0")
            ya = Yfull32.ap()
            for ci in range(NCH):
                cs = slice(512 * ci, 512 * ci + 512)
                pR = [ppool.tile([P, 512], f32, tag="ps", name="ps") for _ in range(2)]
                pU = [ppool.tile([P, 512], f32, tag="ps", name="ps") for _ in range(2)]
                for t in range(KT):
                    rt = rpool.tile([P, 2, 512], f32, tag="rhsh", name="rty", bufs=3)
                    nc.sync.dma_start(rt[:], ya[:, :, t, cs])
                    st = t == 0
                    sp = t == KT - 1
                    rf = lambda j: rt[:, j, :]
                    rr = rf
                    nc.tensor.matmul(pR[0][:], CT32[:, 0, t], rf(0), start=st, stop=False)
                    nc.tensor.matmul(pR[0][:], CT32[:, 2, t], rf(1), start=False, stop=sp)
                    nc.tensor.matmul(pR[1][:], CT32[:, 0, t], rf(1), start=st, stop=False)
                    nc.tensor.matmul(pR[1][:], CT32[:, 1, t], rf(0), start=False, stop=sp)
                    nc.tensor.matmul(pU[0][:], VT[:, 0, t], rr(0), start=st, stop=False)
                    nc.tensor.matmul(pU[0][:], VT[:, 2, t], rr(1), start=False, stop=sp)
                    nc.tensor.matmul(pU[1][:], VT[:, 0, t], rr(1), start=st, stop=False)
                    nc.tensor.matmul(pU[1][:], VT[:, 1, t], rr(0), start=False, stop=sp)
                for j in range(2):
                    nc.vector.tensor_copy(R32[:, j, cs], pR[j][:])
                    nc.vector.tensor_copy(U0[:, j, cs], pU[j][:])
            U0T = transpose_to_weights(U0, tag="T2", dtype=f32r, bufs=1)

            # S1 = R + U0 @ W  (W from SBUF; bf16 halves feed 2 AllGathers)
            S1 = apool.tile([P, 2, N], f32, tag="Sch", bufs=2, name="S1")
            agb_ins = [dram.tile([P, 2, 512], bf16, name="agb_in") for _ in range(NCH)]
            agb_outs = [dram.tile([NC, P, 2, 512], bf16, addr_space="Shared", name="agb_out")
                        for _ in range(NCH)]

            def cb_s1(j, ci, psum):
                cs = slice(512 * ci, 512 * ci + 512)
                nc.vector.tensor_add(S1[:, j, cs], psum[:], R32[:, j, cs])
                stg = apool.tile([P, 512], bf16, tag="stg", bufs=6, name="stg")
                nc.vector.tensor_add(stg[:], psum[:], R32[:, j, cs])
                nc.sync.dma_start(agb_ins[ci][:, j, :], stg[:])

            # tiny rank-sync AG: depends on U0 (ready mid-phase-1), so the
            # ranks re-converge under S1's compute and the real S1 AllGather
            # below runs at data speed instead of absorbing ~30us of skew.
            sync_in = dram.tile([1, 16], f32, name="sync_in")
            sync_out = dram.tile([NC, 16], f32, addr_space="Shared", name="sync_out")
            nc.sync.dma_start(sync_in[:], U0[0:1, 0, 0:16])
            allgather(sync_in, sync_out)
            sync_bk = apool.tile([1, 16], f32, tag="syncbk", bufs=1, name="sync_bk")
            nc.sync.dma_start(sync_bk[:], sync_out[0:1, :])

            wfa = Wfull.ap()
            cgemm(U0T, None, cb_s1, f32r,
                  halves=[lambda t, ci=ci: wfa[:, :, t, 512 * ci:512 * ci + 512]
                          for ci in range(NCH)])
            for ci in range(NCH):
                allgather(agb_ins[ci], agb_outs[ci])

            # ---------------- step 2: S2 = S1 + A1 S1 W1 (bf16 GEMMs) -----
            U1 = apool.tile([P, 2, N], bf16, tag="work", bufs=3, name="U1")
            cgemm(AT1, None, cb_store(U1), bf16,
                  halves=[lambda t, ci=ci: agb_outs[ci][t] for ci in range(NCH)])
            U1T = transpose_to_weights(U1, tag="T2", dtype=bf16, bufs=1)

            S2 = apool.tile([P, 2, N], f32, tag="Sch", bufs=2, name="S2")
            agc_ins = [dram.tile([P, 4, 512], bf16, name="agc_in") for _ in range(NCH)]
            agc_outs = [dram.tile([NC, P, 4, 512], bf16, addr_space="Shared", name="agc_out")
                        for _ in range(NCH)]

            def mk_cb_split(Sdst, addend, ag_ins):
                def cb(j, ci, psum):
                    cs = slice(512 * ci, 512 * ci + 512)
                    nc.vector.tensor_add(Sdst[:, j, cs], psum[:], addend[:, j, cs])
                    h = apool.tile([P, 512], bf16, tag="stg", bufs=6, name="stgh")
                    l = apool.tile([P, 512], bf16, tag="stg", bufs=6, name="stgl")
                    nc.vector.tensor_copy(h[:], Sdst[:, j, cs])
                    nc.vector.tensor_sub(l[:], Sdst[:, j, cs], h[:])
                    nc.sync.dma_start(ag_ins[ci][:, 2 * j, :], h[:])
                    nc.sync.dma_start(ag_ins[ci][:, 2 * j + 1, :], l[:])
                return cb

            cgemm(U1T, None, mk_cb_split(S2, S1, agc_ins), bf16,
                  halves=[lambda t, ci=ci: aga_out[t, :, :, 512 * ci:512 * ci + 512]
                          for ci in range(NCH)])
            for ci in range(NCH):
                allgather(agc_ins[ci], agc_outs[ci])

            # ---------------- polish x2 (split-bf16) ----------------------
            s_half = [lambda t, ci=ci: agc_outs[ci][t] for ci in range(NCH)]
            for it in range(2):
                last = it == 1
                Up = apool.tile([P, 2, N], f32, tag="work", bufs=3, name="Up")
                cgemm_sp(ATspw, None, cb_store(Up), halves=s_half)
                UpT = transpose_to_weights_sp(Up, tag="T1" if it == 0 else "T3",
                                              bufs=2 if it == 0 else 1)

                Sp = apool.tile([P, 2, N], f32, tag="Sch", bufs=2, name="Sp")
                if not last:
                    agd_ins = [dram.tile([P, 4, 512], bf16, name="agd_in") for _ in range(NCH)]
                    agd_outs = [dram.tile([NC, P, 4, 512], bf16, addr_space="Shared",
                                          name="agd_out") for _ in range(NCH)]
                    cgemm_sp(UpT, None, mk_cb_split(Sp, R32, agd_ins), halves=wsp_halves)
                    for ci in range(NCH):
                        allgather(agd_ins[ci], agd_outs[ci])
                    s_half = [lambda t, ci=ci, agd_outs=agd_outs: agd_outs[ci][t]
                              for ci in range(NCH)]
                else:
                    def cb_fin(j, ci, psum):
                        cs = slice(512 * ci, 512 * ci + 512)
                        nc.vector.tensor_add(Sp[:, j, cs], psum[:], R32[:, j, cs])
                        nc.sync.dma_start(out.ap()[j, :, cs], Sp[:, j, cs])

                    cgemm_sp(UpT, None, cb_fin, halves=wsp_halves)

    nc.compile()
    return nc


def _prep_inputs(A, W, C, Y):
    import ml_dtypes
    bf = ml_dtypes.bfloat16

    def full_layout(M, dt=np.float32):
        pl = np.stack([
            M.real.astype(np.float32).astype(dt),
            M.imag.astype(np.float32).astype(dt),
        ])  # [2, 1024, 1024]
        return np.ascontiguousarray(pl.reshape(2, KT, P, N).transpose(2, 0, 1, 3))

    def shard_weights(M, c, dt=np.float32):
        own = M[P * c:P * c + P, :]
        XT = own.T
        r = XT.real.astype(np.float32)
        i = XT.imag.astype(np.float32)
        tr = np.stack([r, i, -i]).astype(dt)  # [3, 1024, 128]
        return np.ascontiguousarray(tr.reshape(3, KT, P, P).transpose(2, 0, 1, 3))

    def split_layout(M):
        # [P, 4, KT, N] bf16: planes (re_h, re_l, im_h, im_l)
        planes = []
        for part in (M.real, M.imag):
            x = part.astype(np.float32)
            h = x.astype(bf)
            l = (x - h.astype(np.float32)).astype(bf)
            planes += [h, l]
        pl = np.stack(planes)  # [4, 1024, 1024]
        return np.ascontiguousarray(pl.reshape(4, KT, P, N).transpose(2, 0, 1, 3))

    def split_shard(M, c):
        # [P, 6, KT, P] bf16: (rh, rl, ih, il, -ih, -il) of own-shard transpose
        XT = M[P * c:P * c + P, :].T
        r = XT.real.astype(np.float32)
        i = XT.imag.astype(np.float32)
        rh = r.astype(bf); rl = (r - rh.astype(np.float32)).astype(bf)
        ih = i.astype(bf); il = (i - ih.astype(np.float32)).astype(bf)
        tr = np.stack([rh, rl, ih, il, -ih, -il])  # [6, 1024, 128]
        return np.ascontiguousarray(tr.reshape(6, KT, P, P).transpose(2, 0, 1, 3))

    Yf = full_layout(Y)
    Cf = full_layout(C)
    Wf = full_layout(W)
    Abf = full_layout(A, bf)
    Wbf = full_layout(W, bf)
    Ws = split_layout(W)
    in_maps = []
    for c in range(NC):
        ATs = shard_weights(A, c)
        in_maps.append({
            "Cfull": Cf, "Wfull": Wf, "Wsp_d": Ws, "Yfull32": Yf,
            "Afull_bf": Abf, "Wfull_bf": Wbf,
            "ATshr": ATs,
            "ATsp": split_shard(A, c),
            "ATsh_bf": shard_weights(A, c, bf),
            "WTsh_bf": shard_weights(W, c, bf),
            "CTsh32": shard_weights(C, c),
        })
    return in_maps


def kernel(A, W, C, Y, _trace=False):
    from concourse import bass_utils

    if "nc" not in _compiled:
        _compiled["nc"] = _build()
    nc = _compiled["nc"]

    in_maps = _prep_inputs(A, W, C, Y)
    res = bass_utils.run_bass_kernel_spmd(
        nc, in_maps, core_ids=list(range(NC)), trace=_trace
    )
    _compiled["last_result"] = res

    full = np.empty((N, N), dtype=np.complex128)
    for c in range(NC):
        o = res.results[c]["out"]
        full[P * c:P * c + P, :] = o[0].astype(np.float64) + 1j * o[1].astype(np.float64)
    return full



# revision 4
# speedup vs baseline: 3.6382x; 3.6382x over previous
"""Stein solver  Lambda - A @ Lambda @ W = C @ Y  on 8 trn2 NeuronCores.

Math: Lambda = sum_k A^k R W^k with R = C@Y; per-step Frobenius contraction
of the series terms is ~0.08, so a 3-term truncation has exact error 5.1e-4
(gate is 2e-2).  Computed as

    S = R + (U0 + T2) @ W,   U0 = (A@C)@Y = A R,   T2 = ((A@A@C)@Y)@W = A^2 R W

which needs NO inter-core collectives at all: every GEMM is either
full-input x full-input or own-rows x full-input.

Distribution: row-sharded over 8 cores, core c owns rows [128c, 128c+128).
Five passes (448 matmuls + 80 transposes per core, vs 1482 in the polish-
heavy predecessor):
  P1: A2 = A@A (Karatsuba, bf16) + V = A@C (4-mult, bf16), fused on one
      streamed-A pass; C resident in SBUF.
  P2: R = C@Y (f32r) + U0 = V@Y (f32r) + V2 = A2@C (bf16), all 4-mult
      (2 PSUM banks each -> 6 of 8 banks), sharing ONE streamed-Y pass.
  P3: U2 = V2@Y (Karatsuba, bf16 Y stream).
  P4: T2 = U2@W (Karatsuba, W resident); combine writes M = U0 + T2.
  P5: S = R + M@W (Karatsuba); combine adds R and DMAs the output.

Precision: R/U0 in f32r (~13 mantissa bits), everything else bf16; the
bf16 terms enter at <= 8e-2 relative scale.  Measured end-to-end error
(CPU simulation of this exact scheme): 6.5e-4.

Karatsuba complex GEMM = 3 real matmuls (planes r, i, r+i) + vector/scalar
combine; 4-mult = 4 real matmuls (planes r, i, -i) with the subtraction
folded into PSUM accumulation (combine is a plain copy).  PSUM-sourced
vector ops always have at most one PSUM operand (copies go via scalar
engine, which is otherwise idle).
"""

import numpy as np

P = 128
N = 1024
KT = N // P          # 8 k-tiles
NC = 8               # cores
NCH = 2              # 512-wide n-chunks per 1024-col output row block
CW = N // NCH        # 512

_compiled = {}


def _build():
    import concourse.mybir as mybir
    import concourse.tile as tile
    from concourse import bacc
    from concourse.masks import make_identity

    f32 = mybir.dt.float32
    f32r = mybir.dt.float32r
    bf16 = mybir.dt.bfloat16

    nc = bacc.Bacc("TRN2", target_bir_lowering=False, debug=False, num_devices=NC)

    # ---- I/O ----
    # full moving matrices laid out [partition, plane, ktile, col]:
    #   X[kt*128+p, c] at [p, j, kt, c]
    # sharded stationary [partition, plane, ktile, m]: (X[own,:].T) blocks
    ATq = nc.dram_tensor("ATq", [P, 4, KT, P], bf16, kind="ExternalInput")     # r,i,r+i,-i
    CTq = nc.dram_tensor("CTq", [P, 3, KT, P], f32r, kind="ExternalInput")     # r,i,-i
    Af = nc.dram_tensor("Af", [P, 3, KT, N], bf16, kind="ExternalInput")       # r,i,r+i
    Cf = nc.dram_tensor("Cf", [P, 2, KT, N], bf16, kind="ExternalInput")       # r,i
    Yfr = nc.dram_tensor("Yfr", [P, 2, KT, N], f32r, kind="ExternalInput")     # r,i
    Yfb = nc.dram_tensor("Yfb", [P, 2, KT, N], bf16, kind="ExternalInput")     # r,i
    Wf = nc.dram_tensor("Wf", [P, 3, KT, N], bf16, kind="ExternalInput")       # r,i,r+i
    out = nc.dram_tensor("out", [2, P, N], f32, kind="ExternalOutput")

    with tile.TileContext(nc) as tc:
        with (
            tc.tile_pool(name="res", bufs=1) as res,          # residents + stationaries
            tc.tile_pool(name="stat", bufs=2) as statp,       # rotating transposed weights
            tc.tile_pool(name="work", bufs=2) as workp,       # rotating bf16 work tiles
            tc.tile_pool(name="mov", bufs=3) as movp,         # streamed moving tiles
            tc.tile_pool(name="tmp", bufs=3) as tmpp,         # combine temporaries
            tc.tile_pool(name="psum", bufs=6, space="PSUM") as ppool,
            tc.tile_pool(name="tpsum", bufs=2, space="PSUM") as tppool,
        ):
            identf = res.tile([P, P], f32, tag="identf")
            make_identity(nc, identf)
            identb = res.tile([P, P], bf16, tag="identb")
            nc.vector.tensor_copy(identb[:], identf[:])

            # PE warmup: keep the tensor engine busy (p-state ramp) while the
            # first DMAs land.  Results are discarded.
            for _ in range(10):
                wtp = tppool.tile([P, P], bf16, tag="tpb", name="wtp")
                nc.tensor.transpose(wtp[:], identb[:], identb[:])

            # small stationaries first, then the C resident (P1 needs both)
            ATw = res.tile([P, 4, KT, P], bf16, tag="ATw")
            nc.sync.dma_start(ATw[:], ATq.ap())
            CTw = res.tile([P, 3, KT, P], f32r, tag="CTw")
            nc.sync.dma_start(CTw[:], CTq.ap())
            Cres = res.tile([P, 2, KT, N], bf16, tag="Cres")
            cfa = Cf.ap()
            for t in range(KT):
                nc.sync.dma_start(Cres[:, :, t, :], cfa[:, :, t, :])

            afa = Af.ap()
            yra = Yfr.ap()
            yba = Yfb.ap()
            wfa = Wf.ap()

            def tr_to_weights(src_bf, wt, mode):
                """src_bf [P,2,N] bf16 -> wt [P,3,KT,P] transposed planes.

                mode 'm4': planes (r, i, -i); mode 'kara': planes (r, i, r+i).
                """
                for t in range(KT):
                    blk = slice(P * t, P * t + P)
                    tpr = tppool.tile([P, P], bf16, tag="tpb", name="tpr")
                    nc.tensor.transpose(tpr[:], src_bf[:, 0, blk], identb[:])
                    nc.scalar.copy(wt[:, 0, t], tpr[:])
                    tpi = tppool.tile([P, P], bf16, tag="tpb", name="tpi")
                    nc.tensor.transpose(tpi[:], src_bf[:, 1, blk], identb[:])
                    nc.scalar.copy(wt[:, 1, t], tpi[:])
                    if mode == "m4":
                        nc.scalar.mul(wt[:, 2, t], tpi[:], -1.0)
                    else:
                        nc.vector.tensor_add(wt[:, 2, t], wt[:, 0, t], tpi[:])

            def kara_combine(pk, cb):
                """pk = [P1, P2, P3] psums; cb(re_src, im_src) with SBUF/psum
                access patterns re = P1-P2, im = P3-P1-P2 (<=1 psum op each)."""
                t1 = tmpp.tile([P, CW], f32, tag="kt", name="t1")
                nc.scalar.copy(t1[:], pk[0][:])
                t3 = tmpp.tile([P, CW], f32, tag="kt", name="t3")
                nc.scalar.copy(t3[:], pk[2][:])
                u = tmpp.tile([P, CW], f32, tag="kt", name="u")
                nc.vector.tensor_sub(u[:], t3[:], t1[:])
                cb(lambda dst: nc.vector.tensor_sub(dst, t1[:], pk[1][:]),
                   lambda dst: nc.vector.tensor_sub(dst, u[:], pk[1][:]))

            # ---------------- P1: A2 = A@A (kara) + V = A@C (4m) ----------
            A2b = workp.tile([P, 2, N], bf16, tag="wb", name="A2b")
            Vb = workp.tile([P, 2, N], bf16, tag="wb", name="Vb")
            for ci in range(NCH):
                cs = slice(CW * ci, CW * ci + CW)
                pa = [ppool.tile([P, CW], f32, tag="ps", name="pa") for _ in range(3)]
                pv = [ppool.tile([P, CW], f32, tag="ps", name="pv") for _ in range(2)]
                for t in range(KT):
                    at = movp.tile([P, 3, CW], bf16, tag="at", name="at")
                    nc.sync.dma_start(at[:], afa[:, :, t, cs])
                    st = t == 0
                    sp = t == KT - 1
                    nc.tensor.matmul(pa[0][:], ATw[:, 0, t], at[:, 0, :], start=st, stop=sp)
                    nc.tensor.matmul(pa[1][:], ATw[:, 1, t], at[:, 1, :], start=st, stop=sp)
                    nc.tensor.matmul(pa[2][:], ATw[:, 2, t], at[:, 2, :], start=st, stop=sp)
                    nc.tensor.matmul(pv[0][:], ATw[:, 0, t], Cres[:, 0, t, cs], start=st, stop=False)
                    nc.tensor.matmul(pv[0][:], ATw[:, 3, t], Cres[:, 1, t, cs], start=False, stop=sp)
                    nc.tensor.matmul(pv[1][:], ATw[:, 0, t], Cres[:, 1, t, cs], start=st, stop=False)
                    nc.tensor.matmul(pv[1][:], ATw[:, 1, t], Cres[:, 0, t, cs], start=False, stop=sp)
                kara_combine(pa, lambda re, im, cs=cs: (re(A2b[:, 0, cs]), im(A2b[:, 1, cs])))
                nc.scalar.copy(Vb[:, 0, cs], pv[0][:])
                nc.scalar.copy(Vb[:, 1, cs], pv[1][:])

            A2T = res.tile([P, 3, KT, P], bf16, tag="A2T")
            tr_to_weights(A2b, A2T, "m4")
            VT = res.tile([P, 3, KT, P], f32r, tag="VT")
            for t in range(KT):
                blk = slice(P * t, P * t + P)
                tpr = tppool.tile([P, P], bf16, tag="tpb", name="vtpr")
                nc.tensor.transpose(tpr[:], Vb[:, 0, blk], identb[:])
                nc.vector.tensor_copy(VT[:, 0, t], tpr[:])
                tpi = tppool.tile([P, P], bf16, tag="tpb", name="vtpi")
                nc.tensor.transpose(tpi[:], Vb[:, 1, blk], identb[:])
                nc.vector.tensor_copy(VT[:, 1, t], tpi[:])
                nc.vector.tensor_scalar_mul(VT[:, 2, t], tpi[:], -1.0)

            # ---------------- P2: R = C@Y + U0 = V@Y + V2 = A2@C (all 4m) -
            R32 = res.tile([P, 2, N], f32, tag="R32")
            U032 = res.tile([P, 2, N], f32, tag="U032")
            V2b = workp.tile([P, 2, N], bf16, tag="wb", name="V2b")
            for ci in range(NCH):
                cs = slice(CW * ci, CW * ci + CW)
                pr = [ppool.tile([P, CW], f32, tag="ps", name="pr") for _ in range(2)]
                pu = [ppool.tile([P, CW], f32, tag="ps", name="pu") for _ in range(2)]
                pv2 = [ppool.tile([P, CW], f32, tag="ps", name="pv2") for _ in range(2)]
                for t in range(KT):
                    yt = movp.tile([P, 2, CW], f32r, tag="yt", name="yt")
                    nc.sync.dma_start(yt[:], yra[:, :, t, cs])
                    st = t == 0
                    sp = t == KT - 1
                    nc.tensor.matmul(pr[0][:], CTw[:, 0, t], yt[:, 0, :], start=st, stop=False)
                    nc.tensor.matmul(pr[0][:], CTw[:, 2, t], yt[:, 1, :], start=False, stop=sp)
                    nc.tensor.matmul(pr[1][:], CTw[:, 0, t], yt[:, 1, :], start=st, stop=False)
                    nc.tensor.matmul(pr[1][:], CTw[:, 1, t], yt[:, 0, :], start=False, stop=sp)
                    nc.tensor.matmul(pu[0][:], VT[:, 0, t], yt[:, 0, :], start=st, stop=False)
                    nc.tensor.matmul(pu[0][:], VT[:, 2, t], yt[:, 1, :], start=False, stop=sp)
                    nc.tensor.matmul(pu[1][:], VT[:, 0, t], yt[:, 1, :], start=st, stop=False)
                    nc.tensor.matmul(pu[1][:], VT[:, 1, t], yt[:, 0, :], start=False, stop=sp)
                    nc.tensor.matmul(pv2[0][:], A2T[:, 0, t], Cres[:, 0, t, cs], start=st, stop=False)
                    nc.tensor.matmul(pv2[0][:], A2T[:, 2, t], Cres[:, 1, t, cs], start=False, stop=sp)
                    nc.tensor.matmul(pv2[1][:], A2T[:, 0, t], Cres[:, 1, t, cs], start=st, stop=False)
                    nc.tensor.matmul(pv2[1][:], A2T[:, 1, t], Cres[:, 0, t, cs], start=False, stop=sp)
                for j in range(2):
                    nc.scalar.copy(R32[:, j, cs], pr[j][:])
                    nc.scalar.copy(U032[:, j, cs], pu[j][:])
                    nc.scalar.copy(V2b[:, j, cs], pv2[j][:])

            # W resident: emitted here so its DMA streams during P2/P3
            # compute, well before its first use in P4.
            Wres = res.tile([P, 3, KT, N], bf16, tag="Wres")
            for t in range(KT):
                nc.sync.dma_start(Wres[:, :, t, :], wfa[:, :, t, :])

            V2T = statp.tile([P, 3, KT, P], bf16, tag="wt", name="V2T")
            tr_to_weights(V2b, V2T, "kara")

            # ---------------- P3: U2 = V2@Y (kara, bf16) ------------------
            U2b = workp.tile([P, 2, N], bf16, tag="wb", name="U2b")
            for ci in range(NCH):
                cs = slice(CW * ci, CW * ci + CW)
                pk = [ppool.tile([P, CW], f32, tag="ps", name="pk") for _ in range(3)]
                for t in range(KT):
                    ybt = movp.tile([P, 2, CW], bf16, tag="ybt", name="ybt", bufs=6)
                    nc.sync.dma_start(ybt[:], yba[:, :, t, cs])
                    ymx = movp.tile([P, CW], bf16, tag="ymx", name="ymx", bufs=2)
                    nc.vector.tensor_add(ymx[:], ybt[:, 0, :], ybt[:, 1, :])
                    st = t == 0
                    sp = t == KT - 1
                    nc.tensor.matmul(pk[0][:], V2T[:, 0, t], ybt[:, 0, :], start=st, stop=sp)
                    nc.tensor.matmul(pk[1][:], V2T[:, 1, t], ybt[:, 1, :], start=st, stop=sp)
                    nc.tensor.matmul(pk[2][:], V2T[:, 2, t], ymx[:], start=st, stop=sp)
                kara_combine(pk, lambda re, im, cs=cs: (re(U2b[:, 0, cs]), im(U2b[:, 1, cs])))

            U2T = statp.tile([P, 3, KT, P], bf16, tag="wt", name="U2T")
            tr_to_weights(U2b, U2T, "kara")

            # ---------------- P4: T2 = U2@W; M = U0 + T2 ------------------
            Mb = workp.tile([P, 2, N], bf16, tag="wb", name="Mb")
            for ci in range(NCH):
                cs = slice(CW * ci, CW * ci + CW)
                pk = [ppool.tile([P, CW], f32, tag="ps", name="pt") for _ in range(3)]
                for t in range(KT):
                    st = t == 0
                    sp = t == KT - 1
                    nc.tensor.matmul(pk[0][:], U2T[:, 0, t], Wres[:, 0, t, cs], start=st, stop=sp)
                    nc.tensor.matmul(pk[1][:], U2T[:, 1, t], Wres[:, 1, t, cs], start=st, stop=sp)
                    nc.tensor.matmul(pk[2][:], U2T[:, 2, t], Wres[:, 2, t, cs], start=st, stop=sp)

                def cbm(re, im, cs=cs):
                    rr = tmpp.tile([P, CW], f32, tag="kt", name="rr")
                    re(rr[:])
                    nc.vector.tensor_add(Mb[:, 0, cs], rr[:], U032[:, 0, cs])
                    ii = tmpp.tile([P, CW], f32, tag="kt", name="ii")
                    im(ii[:])
                    nc.vector.tensor_add(Mb[:, 1, cs], ii[:], U032[:, 1, cs])

                kara_combine(pk, cbm)

            MT = statp.tile([P, 3, KT, P], bf16, tag="wt", name="MT")
            tr_to_weights(Mb, MT, "kara")

            # ---------------- P5: S = R + M@W; write out ------------------
            oa = out.ap()
            for ci in range(NCH):
                cs = slice(CW * ci, CW * ci + CW)
                pk = [ppool.tile([P, CW], f32, tag="ps", name="pf") for _ in range(3)]
                for t in range(KT):
                    st = t == 0
                    sp = t == KT - 1
                    nc.tensor.matmul(pk[0][:], MT[:, 0, t], Wres[:, 0, t, cs], start=st, stop=sp)
                    nc.tensor.matmul(pk[1][:], MT[:, 1, t], Wres[:, 1, t, cs], start=st, stop=sp)
                    nc.tensor.matmul(pk[2][:], MT[:, 2, t], Wres[:, 2, t, cs], start=st, stop=sp)

                def cbf(re, im, cs=cs, ci=ci):
                    for j, part in ((0, re), (1, im)):
                        pp = tmpp.tile([P, CW], f32, tag="kt", name="pp")
                        part(pp[:])
                        og = tmpp.tile([P, CW], f32, tag="og", name="og", bufs=2)
                        nc.vector.tensor_add(og[:], pp[:], R32[:, j, cs])
                        nc.sync.dma_start(oa[j, :, cs], og[:])

                kara_combine(pk, cbf)

    nc.compile()
    return nc


def _prep_inputs(A, W, C, Y):
    import ml_dtypes
    bf = ml_dtypes.bfloat16

    def full_layout(M, planes, dt):
        pl = np.stack(planes)  # [p, 1024, 1024]
        return np.ascontiguousarray(
            pl.reshape(len(planes), KT, P, N).transpose(2, 0, 1, 3).astype(dt))

    def shard_layout(M, c, planes_fn, dt):
        XT = M[P * c:P * c + P, :].T
        r = XT.real.astype(np.float32)
        i = XT.imag.astype(np.float32)
        pl = np.stack(planes_fn(r, i))  # [p, 1024, 128]
        npl = pl.shape[0]
        return np.ascontiguousarray(
            pl.reshape(npl, KT, P, P).transpose(2, 0, 1, 3).astype(dt))

    def re_im(M):
        return M.real.astype(np.float32), M.imag.astype(np.float32)

    Ar, Ai = re_im(A)
    Cr, Ci = re_im(C)
    Yr, Yi = re_im(Y)
    Wr, Wi = re_im(W)

    Af = full_layout(A, [Ar, Ai, Ar + Ai], bf)
    Cfull = full_layout(C, [Cr, Ci], bf)
    Yfr = full_layout(Y, [Yr, Yi], np.float32)
    Yfb = full_layout(Y, [Yr, Yi], bf)
    Wfull = full_layout(W, [Wr, Wi, Wr + Wi], bf)

    in_maps = []
    for c in range(NC):
        in_maps.append({
            "ATq": shard_layout(A, c, lambda r, i: [r, i, r + i, -i], bf),
            "CTq": shard_layout(C, c, lambda r, i: [r, i, -i], np.float32),
            "Af": Af, "Cf": Cfull, "Yfr": Yfr, "Yfb": Yfb, "Wf": Wfull,
        })
    return in_maps


def kernel(A, W, C, Y, _trace=False):
    from concourse import bass_utils

    if "nc" not in _compiled:
        _compiled["nc"] = _build()
    nc = _compiled["nc"]

    in_maps = _prep_inputs(A, W, C, Y)
    res = bass_utils.run_bass_kernel_spmd(
        nc, in_maps, core_ids=list(range(NC)), trace=_trace
    )
    _compiled["last_result"] = res

    full = np.empty((N, N), dtype=np.complex128)
    for c in range(NC):
        o = res.results[c]["out"]
        full[P * c:P * c + P, :] = o[0].astype(np.float64) + 1j * o[1].astype(np.float64)
    return full
